# revision 22
# baseline (speedup 1.0000x reference)
"""Trainium2 Bass kernel for nn_Metric_42674795053594 (Relation Network loss).

Self-contained: hardcodes all shapes. Shards batch b=8 across 8 NeuronCores
(1 episode/core), replicates params, uses AllReduce for training-mode
BatchNorm statistics that couple all episodes.
"""
import sys, os
sys.path.insert(0, '/opt/trn_rl_repo')
import numpy as np
import ml_dtypes

import concourse.bass as bass
import concourse.mybir as mybir
import concourse.tile as tile
from concourse import bacc
from concourse.bass_utils import run_bass_kernel_spmd

F32 = mybir.dt.float32
BF16 = mybir.dt.bfloat16
AF = mybir.ActivationFunctionType
ALU = mybir.AluOpType
AX = mybir.AxisListType

EPS = 1e-5
NCORES = 8
S, Q = 5, 30
NPAIR = 18          # 36 image slots (5 sup + 30 qry + 1 pad) packed 2/partition-half
IMGW = 84
PLANE = 7232        # padded per-channel plane stride (>= 7056 + 170)
W1 = 7056           # conv1 output width (84*84)
PW1, PW2 = 1681, 361   # pooled widths: 41*41, 19*19
PAD1, PAD2 = 88, 40    # shift-overshoot pads (2*41+2+chunk slack, 2*19+2)
GROWS = Q * S * 81     # 12150 pairwise rows per core
CHUNK = 486            # 6 blocks of 81, <= 512

# conv2 input pooled1 is stored in this dtype (bf16 halves SBUF + 2x PE for conv2)
POOL1_DT = BF16


def _bn_scalar_ops(nc, pool, s_ap, q_ap, n_elems, g_ap, b_ap, sc_out, sh_out, eps_ap, tag):
    """Given sum (s_ap) and sumsq (q_ap) APs [P,1], counts, gamma/beta APs,
    write scale into sc_out and shift into sh_out ([P,1])."""
    P = s_ap.shape[0]
    t = pool.tile([128, 4], F32, tag=f"bns_{tag}")
    mean, ex2, var, m2 = t[:P, 0:1], t[:P, 1:2], t[:P, 2:3], t[:P, 3:4]
    nc.vector.tensor_scalar_mul(mean, s_ap, 1.0 / n_elems)
    nc.vector.tensor_scalar_mul(ex2, q_ap, 1.0 / n_elems)
    nc.vector.tensor_tensor(m2, mean, mean, ALU.mult)
    nc.vector.tensor_tensor(var, ex2, m2, ALU.subtract)
    # sd = sqrt(var + eps); inv = 1/sd
    nc.scalar.activation(var, var, AF.Sqrt, bias=eps_ap)
    nc.vector.reciprocal(var, var)
    nc.vector.tensor_tensor(sc_out, g_ap, var, ALU.mult)
    nc.vector.tensor_tensor(m2, mean, sc_out, ALU.mult)
    nc.vector.tensor_tensor(sh_out, b_ap, m2, ALU.subtract)


def build_nc(n_cores=NCORES, debug=False):
    nc = bacc.Bacc("TRN2", target_bir_lowering=False, debug=False, num_devices=n_cores)
    RG = [list(range(n_cores))]

    # ---------------- I/O ----------------
    imgs_d = nc.dram_tensor("imgs", [36, 3, PLANE], BF16, kind="ExternalInput")
    w1t_d = nc.dram_tensor("w1t", [54, 128], BF16, kind="ExternalInput")
    wct_d = nc.dram_tensor("wct", [128, 3, 9, 128], F32, kind="ExternalInput")
    bng_d = nc.dram_tensor("bng", [64, 4], F32, kind="ExternalInput")
    bnb_d = nc.dram_tensor("bnb", [64, 4], F32, kind="ExternalInput")
    gw1s_d = nc.dram_tensor("gw1s", [66, 256], F32, kind="ExternalInput")
    gw1q_d = nc.dram_tensor("gw1q", [66, 256], F32, kind="ExternalInput")
    gb1_d = nc.dram_tensor("gb1t", [128, 2], F32, kind="ExternalInput")
    gwt_d = nc.dram_tensor("gwt", [128, 3, 2, 256], F32, kind="ExternalInput")
    gbt_d = nc.dram_tensor("gbt", [128, 3, 2], F32, kind="ExternalInput")
    fwt_d = nc.dram_tensor("fwt", [128, 2, 2, 256], F32, kind="ExternalInput")
    fbt_d = nc.dram_tensor("fbt", [128, 2, 2], F32, kind="ExternalInput")
    fw3_d = nc.dram_tensor("fw3t", [128, 2, 64], F32, kind="ExternalInput")
    fb3_d = nc.dram_tensor("fb3t", [64, 1], F32, kind="ExternalInput")
    fw4_d = nc.dram_tensor("fw4t", [64, 1], F32, kind="ExternalInput")
    fb4_d = nc.dram_tensor("fb4t", [1, 1], F32, kind="ExternalInput")
    fbng_d = nc.dram_tensor("fbng", [128, 2], F32, kind="ExternalInput")
    fbnb_d = nc.dram_tensor("fbnb", [128, 2], F32, kind="ExternalInput")
    coord45_d = nc.dram_tensor("coord45", [2, 45], F32, kind="ExternalInput")
    coord270_d = nc.dram_tensor("coord270", [2, 270], F32, kind="ExternalInput")
    lbl_d = nc.dram_tensor("lbl", [1, 150], F32, kind="ExternalInput")
    apmask_d = nc.dram_tensor("apmask", [1, 150], F32, kind="ExternalInput")

    loss_d = nc.dram_tensor("loss_part", [1, 1], F32, kind="ExternalOutput")
    if debug:
        feats_dbg_d = nc.dram_tensor("feats_dbg", [66, 324], F32, kind="ExternalOutput")
        dist_dbg_d = nc.dram_tensor("dist_dbg", [1, 150], F32, kind="ExternalOutput")
        xf_dbg_d = nc.dram_tensor("xf_dbg", [128, 2, 150], F32, kind="ExternalOutput")

    with tile.TileContext(nc) as tc:
        with (
            tc.tile_pool(name="pers", bufs=1) as pers,
            tc.tile_pool(name="dram", bufs=1, space="DRAM") as dram,
        ):
            # ---------------- load persistent weights ----------------
            w1t = pers.tile([54, 128], BF16)
            nc.sync.dma_start(w1t[:], w1t_d[:])
            wct = pers.tile([128, 3, 9, 128], F32)
            nc.sync.dma_start(wct[:], wct_d[:])
            wct2b = pers.tile([128, 9, 128], POOL1_DT)   # conv2 weights in pooled1 dtype
            nc.vector.tensor_copy(wct2b[:], wct[:, 0])
            bng = pers.tile([64, 4], F32)
            nc.sync.dma_start(bng[:], bng_d[:])
            bnb = pers.tile([64, 4], F32)
            nc.sync.dma_start(bnb[:], bnb_d[:])
            gw1s = pers.tile([66, 256], F32)
            nc.sync.dma_start(gw1s[:], gw1s_d[:])
            gw1q = pers.tile([66, 256], F32)
            nc.sync.dma_start(gw1q[:], gw1q_d[:])
            gb1 = pers.tile([128, 2], F32)
            nc.sync.dma_start(gb1[:], gb1_d[:])
            gwt = pers.tile([128, 3, 2, 256], F32)
            nc.sync.dma_start(gwt[:], gwt_d[:])
            gbt = pers.tile([128, 3, 2], F32)
            nc.sync.dma_start(gbt[:], gbt_d[:])
            fwt = pers.tile([128, 2, 2, 256], F32)
            nc.sync.dma_start(fwt[:], fwt_d[:])
            fbt = pers.tile([128, 2, 2], F32)
            nc.sync.dma_start(fbt[:], fbt_d[:])
            fw3 = pers.tile([128, 2, 64], F32)
            nc.sync.dma_start(fw3[:], fw3_d[:])
            fb3 = pers.tile([64, 1], F32)
            nc.sync.dma_start(fb3[:], fb3_d[:])
            fw4 = pers.tile([64, 1], F32)
            nc.sync.dma_start(fw4[:], fw4_d[:])
            fb4 = pers.tile([1, 1], F32)
            nc.sync.dma_start(fb4[:], fb4_d[:])
            fbng = pers.tile([128, 2], F32)
            nc.sync.dma_start(fbng[:], fbng_d[:])
            fbnb = pers.tile([128, 2], F32)
            nc.sync.dma_start(fbnb[:], fbnb_d[:])
            lbl_sb = pers.tile([1, 150], F32)
            nc.sync.dma_start(lbl_sb[:], lbl_d[:])
            apmask_sb = pers.tile([1, 150], F32)
            nc.sync.dma_start(apmask_sb[:], apmask_d[:])

            epsc = pers.tile([128, 1], F32)
            nc.gpsimd.memset(epsc[:], EPS)
            margin = pers.tile([1, 1], F32)
            nc.gpsimd.memset(margin[:], 0.2)

            # persistent activations
            pooled2 = pers.tile([128, NPAIR * PW2 + PAD2], F32)
            nc.gpsimd.memset(pooled2[:, NPAIR * PW2:], 0.0)
            feats = pers.tile([66, 324], F32)
            nc.sync.dma_start(feats[64:66, 0:45], coord45_d[:])
            nc.sync.dma_start(feats[64:66, 45:315], coord270_d[:])
            # BN scale/shift per conv layer: [128, 2] (col0 sup, col1 qry)
            sc_t = [pers.tile([128, 2], F32, tag=f"sc{l}", name=f"sc{l}") for l in range(4)]
            sh_t = [pers.tile([128, 2], F32, tag=f"sh{l}", name=f"sh{l}") for l in range(4)]
            xf = pers.tile([128, 2, 150], F32)

            # ---- helper: stats + allreduce + scale/shift for one conv layer ----
            def conv_bn(layer, buf, Wimg, valid_view_fn, sup_elems, qry_elems, cc_tag, sum_axis=AX.X):
                """buf: [128, NPAIR*Wimg(+pad)]; valid_view_fn(half_slice, p0, np_)
                returns the valid-region AP for pairs [p0, p0+np_).
                Computes region sums + per-pair sumsq, allreduces, fills
                sc_t[layer], sh_t[layer]."""
                st = pers.tile([128, 8], F32, tag=f"stt{layer}")
                # per-pair sums + sumsq (overlap with conv); region-combine later
                sumacc = pers.tile([128, NPAIR], F32, tag=f"sma{layer}")
                sqacc = pers.tile([128, NPAIR], F32, tag=f"sqa{layer}")
                sqs = pers.tile([128, 512], F32, tag="sq_scratch")
                for p in range(NPAIR):
                    v = valid_view_fn(slice(0, 128), p, 1)
                    nc.vector.reduce_sum(sumacc[:, p:p + 1], v, axis=sum_axis)
                    n_el = v.free_size()
                    # chunk the square dump through the 512-wide scratch
                    if n_el <= 512:
                        nc.scalar.activation(sqs[:, :n_el], v, AF.Square,
                                             accum_out=sqacc[:, p:p + 1])
                    else:
                        # pooled maps are contiguous; square in 512-col chunks
                        flat = buf[:, p * Wimg:(p + 1) * Wimg]
                        nsub = (Wimg + 511) // 512
                        part = pers.tile([128, 4], F32, tag=f"sqp{layer}")
                        for sub in range(nsub):
                            a, b = sub * 512, min((sub + 1) * 512, Wimg)
                            nc.scalar.activation(sqs[:, :b - a], flat[:, a:b], AF.Square,
                                                 accum_out=part[:, sub % 4:sub % 4 + 1])
                        # sum the partials (nsub<=4)
                        nc.vector.reduce_sum(sqacc[:, p:p + 1], part[:, :nsub], axis=AX.X)
                nc.vector.reduce_sum(st[0:64, 0:1], sumacc[0:64, 0:5], axis=AX.X)
                nc.vector.reduce_sum(st[0:64, 1:2], sumacc[0:64, 5:18], axis=AX.X)
                nc.vector.reduce_sum(st[64:128, 1:2], sumacc[64:128, :], axis=AX.X)
                nc.vector.reduce_sum(st[0:64, 2:3], sqacc[0:64, 0:5], axis=AX.X)
                nc.vector.reduce_sum(st[0:64, 3:4], sqacc[0:64, 5:18], axis=AX.X)
                nc.vector.reduce_sum(st[64:128, 3:4], sqacc[64:128, :], axis=AX.X)
                # pack [64, 4]: sup_sum, sup_sq, qry_sum_partial(top), qry_sq_partial(top)
                pk = pers.tile([64, 8], F32, tag=f"pk{layer}")
                nc.vector.tensor_copy(pk[:, 0:1], st[0:64, 0:1])
                nc.vector.tensor_copy(pk[:, 1:2], st[0:64, 2:3])
                # qry partials: top + bottom (bottom moved down via DMA)
                nc.sync.dma_start(pk[:, 4:5], st[64:128, 1:2])
                nc.sync.dma_start(pk[:, 5:6], st[64:128, 3:4])
                nc.vector.tensor_tensor(pk[:, 2:3], st[0:64, 1:2], pk[:, 4:5], ALU.add)
                nc.vector.tensor_tensor(pk[:, 3:4], st[0:64, 3:4], pk[:, 5:6], ALU.add)
                # allgather [64,4] -> [64*N,4], then local sum (AG floor < AR floor)
                bin_ = dram.tile([64, 4], F32, tag=f"ccin{cc_tag}")
                bout = dram.tile([64 * n_cores, 4], F32, tag=f"ccout{cc_tag}")
                nc.gpsimd.dma_start(bin_[:], pk[:, 0:4])
                nc.gpsimd.collective_compute("AllGather", ALU.bypass, replica_groups=RG,
                                             ins=[bin_.opt()], outs=[bout.opt()])
                gat = pers.tile([64, 4 * n_cores], F32, tag=f"gat{layer}")
                nc.sync.dma_start(gat[:], bout.rearrange("(r p) f -> p r f", p=64))
                red = pers.tile([64, 4], F32, tag=f"red{layer}")
                nc.vector.reduce_sum(red[:], gat.rearrange("p (r f) -> p f r", r=n_cores),
                                     axis=AX.X)
                _bn_scalar_ops(nc, pers, red[:, 0:1], red[:, 1:2], sup_elems,
                               bng[:, layer:layer + 1], bnb[:, layer:layer + 1],
                               sc_t[layer][0:64, 0:1], sh_t[layer][0:64, 0:1], epsc[0:64], f"s{layer}")
                _bn_scalar_ops(nc, pers, red[:, 2:3], red[:, 3:4], qry_elems,
                               bng[:, layer:layer + 1], bnb[:, layer:layer + 1],
                               sc_t[layer][0:64, 1:2], sh_t[layer][0:64, 1:2], epsc[0:64], f"q{layer}")
                # replicate qry scale/shift to bottom partitions
                nc.sync.dma_start(sc_t[layer][64:128, 1:2], sc_t[layer][0:64, 1:2])
                nc.sync.dma_start(sh_t[layer][64:128, 1:2], sh_t[layer][0:64, 1:2])

            def bn_apply_pairs(layer, view_fn, out_view_fn=None):
                # per-pair BN+relu so the next conv layer pipelines behind it
                for p in range(NPAIR):
                    top_in = view_fn(slice(0, 64), p, 1)
                    bot_in = view_fn(slice(64, 128), p, 1)
                    top_out = out_view_fn(slice(0, 64), p, 1) if out_view_fn else top_in
                    bot_out = out_view_fn(slice(64, 128), p, 1) if out_view_fn else bot_in
                    col = 0 if p < 5 else 1  # top half holds support for pairs 0-4
                    nc.scalar.activation(top_out, top_in, AF.Relu,
                                         bias=sh_t[layer][0:64, col:col + 1],
                                         scale=sc_t[layer][0:64, col:col + 1])
                    nc.scalar.activation(bot_out, bot_in, AF.Relu,
                                         bias=sh_t[layer][64:128, 1:2],
                                         scale=sc_t[layer][64:128, 1:2])

            # ================= PHASE 1: conv1 + pool + BN, conv2 + pool + BN =================
            with (
                tc.tile_pool(name="ph1", bufs=1) as ph1,
                tc.tile_pool(name="ph1b", bufs=3) as ph1b,
                tc.tile_pool(name="ph1ps", bufs=6, space="PSUM") as psum,
            ):
                pooled1 = ph1.tile([128, NPAIR * PW1 + PAD1], POOL1_DT)
                nc.gpsimd.memset(pooled1[:, NPAIR * PW1:], 0.0)

                for p in range(NPAIR):
                    in27 = ph1b.tile([54, W1], BF16, tag="in27")
                    for half, img in ((0, p), (1, 18 + p)):
                        for kx in range(3):
                            src_ap = bass.AP(tensor=imgs_d.ap().tensor,
                                             offset=img * 3 * PLANE + kx * IMGW,
                                             ap=[[1, 3], [PLANE, 3], [1, W1]])
                            r0 = half * 27 + kx * 9
                            nc.sync.dma_start(in27[r0:r0 + 9, :], src_ap)
                    # 14 chunks of 504 cols = 6 input rows each; pool 2x2 from PSUM
                    for c in range(14):
                        a = c * 504
                        ps = psum.tile([128, 512], F32, tag="cps")
                        nc.tensor.matmul(ps[:, :504], w1t[:, :], in27[:, a:a + 504])
                        orows = 3 if c < 13 else 2
                        v5 = ps[:, :504].rearrange("p (r c) -> p r c", r=6)[:, :, 0:82] \
                            .rearrange("p (R rp) (C cp) -> p R C rp cp", rp=2, cp=2)[:, :orows]
                        dst = pooled1[:, p * PW1 + 3 * c * 41: p * PW1 + (3 * c + orows) * 41]
                        nc.vector.tensor_reduce(dst.rearrange("p (r c) -> p r c", r=orows),
                                                v5, axis=AX.XY, op=ALU.max)

                # ---- L1 BN ----
                def l1_view(hs, p0, np_):
                    return pooled1[hs, p0 * PW1:(p0 + np_) * PW1]
                conv_bn(0, pooled1, PW1, l1_view, NCORES * 5 * PW1, NCORES * 30 * PW1, 0)
                bn_apply_pairs(0, l1_view)
                nc.gpsimd.memset(pooled1[64:128, 17 * PW1:18 * PW1], 0.0)  # pad img

                # ---- conv2 (bf16 in, fp32 psum) + pool (41->39 valid ->19) ----
                c2widths = [492, 492, 492, 205]
                for p in range(NPAIR):
                    base = p * PW1
                    pstiles = [psum.tile([128, 512], F32, tag="cps", name=f"c2ps{_i}") for _i in range(4)]
                    for j in range(9):
                        sh = (j // 3) * 41 + (j % 3)
                        for c in range(4):
                            a = c * 492
                            w = c2widths[c]
                            nc.tensor.matmul(
                                pstiles[c][:, :w], wct2b[:, j, :],
                                pooled1[:, base + a + sh: base + a + sh + w],
                                start=(j == 0), stop=(j == 8))
                    for c in range(4):
                        orows = 6 if c < 3 else 1
                        inrows = 12 if c < 3 else 5
                        v5 = pstiles[c][:, :inrows * 41].rearrange("p (r c) -> p r c", r=inrows)[:, :2 * orows, 0:38] \
                            .rearrange("p (R rp) (C cp) -> p R C rp cp", rp=2, cp=2)
                        dst = pooled2[:, p * PW2 + 6 * c * 19: p * PW2 + (6 * c + orows) * 19]
                        nc.vector.tensor_reduce(dst.rearrange("p (r c) -> p r c", r=orows),
                                                v5, axis=AX.XY, op=ALU.max)

            # ---- L2 BN ----
            def l2_view(hs, p0, np_):
                return pooled2[hs, p0 * PW2:(p0 + np_) * PW2]
            conv_bn(1, pooled2, PW2, l2_view, NCORES * 5 * PW2, NCORES * 30 * PW2, 1)
            bn_apply_pairs(1, l2_view)
            nc.gpsimd.memset(pooled2[64:128, 17 * PW2:18 * PW2], 0.0)

            # ================= PHASE 2: conv3, conv4, avgpool =================
            with (
                tc.tile_pool(name="ph2", bufs=1) as ph2,
                tc.tile_pool(name="ph2ps", bufs=8, space="PSUM") as psum,
            ):
                c3buf = ph2.tile([128, NPAIR * PW2 + PAD2], F32)
                nc.gpsimd.memset(c3buf[:, NPAIR * PW2:], 0.0)
                PW3 = 289  # 17*17 repacked width for conv4
                c17 = ph2.tile([128, NPAIR * PW3 + 36], F32)
                nc.gpsimd.memset(c17[:, NPAIR * PW3:], 0.0)
                c4buf = ph2.tile([128, NPAIR * PW3], F32)

                def conv_layer(src, dstbuf, lidx, W, Wo):
                    # src [128, NPAIR*W(+pad)] W-wide rows; out Wo cols per img
                    for pb in range(0, NPAIR, 4):
                        pe = min(pb + 4, NPAIR)
                        pst = {pp: psum.tile([128, 512], F32, tag="cps", name=f"c34ps{pp}") for pp in range(pb, pe)}
                        rowlen = int(round(W ** 0.5))
                        for j in range(9):
                            sh = (j // 3) * rowlen + (j % 3)
                            for pp in range(pb, pe):
                                base = pp * W
                                nc.tensor.matmul(
                                    pst[pp][:, :Wo], wct[:, lidx, j, :],
                                    src[:, base + sh: base + sh + Wo],
                                    start=(j == 0), stop=(j == 8))
                        for pp in range(pb, pe):
                            nc.scalar.activation(dstbuf[:, pp * Wo:(pp + 1) * Wo],
                                                 pst[pp][:, :Wo], AF.Copy)

                conv_layer(pooled2, c3buf, 1, PW2, PW2)

                def l3_view(hs, p0, np_):
                    return c3buf[hs, p0 * PW2:(p0 + np_) * PW2].rearrange(
                        "p (i r c) -> p i r c", r=19, c=19)[:, :, 0:17, 0:17]
                def c17_view(hs, p0, np_):
                    return c17[hs, p0 * PW3:(p0 + np_) * PW3].rearrange(
                        "p (i r c) -> p i r c", r=17, c=17)
                conv_bn(2, c3buf, PW2, l3_view, NCORES * 5 * 289, NCORES * 30 * 289, 2, sum_axis=AX.XYZ)
                bn_apply_pairs(2, l3_view, c17_view)
                nc.gpsimd.memset(c17[64:128, 17 * PW3:18 * PW3], 0.0)

                conv_layer(c17, c4buf, 2, PW3, PW3)

                def l4_view(hs, p0, np_):
                    return c4buf[hs, p0 * PW3:(p0 + np_) * PW3].rearrange(
                        "p (i r c) -> p i r c", r=17, c=17)[:, :, 0:15, 0:15]
                conv_bn(3, c4buf, PW3, l4_view, NCORES * 5 * 225, NCORES * 30 * 225, 3, sum_axis=AX.XYZ)
                bn_apply_pairs(3, l4_view)

                # ---- avgpool 5x5 -> [64, 9] per image ----
                featsB = ph2.tile([128, 162], F32)
                ptmp = ph2.tile([128, 45], F32, tag="ptmp")
                for i in range(36):
                    half, p = (0, i) if i < 18 else (1, i - 18)
                    if half == 1 and p == 17:
                        continue  # pad image unused
                    hs = slice(half * 64, half * 64 + 64)
                    base = p * PW3
                    # stage 1: sum over kcol(5): [64, 15(17), 3(5), 5(1)] -> [64, 45]
                    v1 = c4buf[hs, base:base + PW3].rearrange(
                        "p (r c) -> p r c", r=17)[:, 0:15, 0:15].rearrange(
                        "p r (oc k) -> p r oc k", oc=3)
                    nc.vector.reduce_sum(ptmp[hs, :].rearrange("p (r oc) -> p r oc", r=15),
                                         v1, axis=AX.X)
                    # stage 2: sum over krow(5): t[r, oc] r=5R+kr -> [64, 9]
                    v2 = ptmp[hs, :].rearrange("p (R k oc) -> p R oc k", R=3, k=5, oc=3)
                    if half == 0:
                        dst = feats[0:64, i * 9:(i + 1) * 9].rearrange("p (R oc) -> p R oc", R=3)
                        nc.vector.reduce_sum(dst, v2, axis=AX.X)
                    else:
                        dstB = featsB[hs, p * 9:(p + 1) * 9].rearrange("p (R oc) -> p R oc", R=3)
                        nc.vector.reduce_sum(dstB, v2, axis=AX.X)
                nc.sync.dma_start(feats[0:64, 162:315], featsB[64:128, 0:153])
                nc.vector.tensor_scalar_mul(feats[0:64, 0:315], feats[0:64, 0:315], 1.0 / 25.0)

            if debug:
                nc.sync.dma_start(feats_dbg_d[:], feats[:])

            # ================= PHASE 3: pairwise g-MLP + f-MLP + loss =================
            with (
                tc.tile_pool(name="ph3", bufs=3) as ph3,
                tc.tile_pool(name="ph3psum", bufs=2, space="PSUM") as ps3,
                tc.tile_pool(name="ph3psg", bufs=2, space="PSUM") as psg,
            ):
                # A[mb] [128, 45], B[mb] [128, 270]
                A = [ph3.tile([128, 45], F32, tag=f"A{m}", name=f"A{m}") for m in range(2)]
                Bq = [ph3.tile([128, 270], F32, tag=f"B{m}", name=f"B{m}") for m in range(2)]
                for m in range(2):
                    pa = ps3.tile([128, 512], F32, tag="abps")
                    nc.tensor.matmul(pa[:, 0:45], gw1s[:, m * 128:(m + 1) * 128], feats[:, 0:45])
                    nc.scalar.activation(A[m][:], pa[:, 0:45], AF.Identity, bias=gb1[:, m:m + 1])
                    pb = ps3.tile([128, 512], F32, tag="abps")
                    nc.tensor.matmul(pb[:, 0:270], gw1q[:, m * 128:(m + 1) * 128], feats[:, 45:315])
                    nc.scalar.activation(Bq[m][:], pb[:, 0:270], AF.Copy)

                QCH = 405  # one query row-block: 5 s * 81 xy
                for qp in range(0, Q, 2):
                    qpair = (qp, qp + 1)
                    h = {}
                    for qi, q in enumerate(qpair):
                        x1 = [ph3.tile([128, QCH], F32, tag=f"x1_{qi}_{k}", name=f"x1_{qi}_{k}")
                              for k in range(2)]
                        for k in range(2):
                            a_in = A[k][:, :, None].to_broadcast((128, 45, 9))
                            b_in = Bq[k][:, None, q * 9:q * 9 + 9].to_broadcast((128, 45, 9))
                            out = x1[k][:].rearrange("p (sx y) -> p sx y", y=9)
                            nc.vector.tensor_tensor(out, a_in, b_in, ALU.add)
                            nc.scalar.activation(x1[k][:], x1[k][:], AF.Relu)
                        h[qi] = x1
                    for l in range(3):
                        hn = {qi: [ph3.tile([128, QCH], F32, tag=f"h{qi}_{l}_{m}", name=f"h{qi}_{l}_{m}")
                                   for m in range(2)] for qi in range(2)}
                        for m in range(2):
                            ps = {qi: psg.tile([128, 512], F32, tag=f"gps{qi}", name=f"gps{qi}")
                                  for qi in range(2)}
                            for ks in range(2):
                                for qi in range(2):
                                    nc.tensor.matmul(ps[qi][:, :QCH],
                                                     gwt[:, l, ks, m * 128:(m + 1) * 128],
                                                     h[qi][ks][:],
                                                     start=(ks == 0), stop=(ks == 1))
                            for qi in range(2):
                                nc.scalar.activation(hn[qi][m][:], ps[qi][:, :QCH], AF.Relu,
                                                     bias=gbt[:, l, m:m + 1])
                        h = hn
                    for qi, q in enumerate(qpair):
                        for m in range(2):
                            nc.vector.reduce_sum(xf[:, m, q * 5:(q + 1) * 5],
                                                 h[qi][m].rearrange("p (b e) -> p b e", e=81), axis=AX.X)

                # ---- fbn stats + allreduce ----
                fst = ph3.tile([128, 4], F32, tag="fst")
                sqf = ph3.tile([128, 150], F32, tag="sqf")
                for m in range(2):
                    nc.vector.reduce_sum(fst[:, 2 * m:2 * m + 1], xf[:, m], axis=AX.X)
                    nc.scalar.activation(sqf[:], xf[:, m], AF.Square,
                                         accum_out=fst[:, 2 * m + 1:2 * m + 2])
                fbin = dram.tile([128, 4], F32, tag="ccfin")
                fbout = dram.tile([128 * n_cores, 4], F32, tag="ccfout")
                nc.gpsimd.dma_start(fbin[:], fst[:])
                nc.gpsimd.collective_compute("AllGather", ALU.bypass, replica_groups=RG,
                                             ins=[fbin.opt()], outs=[fbout.opt()])
                fgat = ph3.tile([128, 4 * n_cores], F32, tag="fgat")
                nc.sync.dma_start(fgat[:], fbout.rearrange("(r p) f -> p r f", p=128))
                fred = ph3.tile([128, 4], F32, tag="fred")
                nc.vector.reduce_sum(fred[:], fgat.rearrange("p (r f) -> p f r", r=n_cores),
                                     axis=AX.X)
                fsc = ph3.tile([128, 2], F32, tag="fsc")
                fsh = ph3.tile([128, 2], F32, tag="fsh")
                for m in range(2):
                    _bn_scalar_ops(nc, ph3, fred[:, 2 * m:2 * m + 1], fred[:, 2 * m + 1:2 * m + 2],
                                   1200.0, fbng[:, m:m + 1], fbnb[:, m:m + 1],
                                   fsc[:, m:m + 1], fsh[:, m:m + 1], epsc[:], f"f{m}")

                if debug:
                    nc.sync.dma_start(xf_dbg_d[:], xf[:])

                # ---- f-MLP on [*, 150] ----
                y = [ph3.tile([128, 150], F32, tag=f"y{m}", name=f"y{m}") for m in range(2)]
                for m in range(2):
                    nc.scalar.activation(y[m][:], xf[:, m], AF.Identity,
                                         bias=fsh[:, m:m + 1], scale=fsc[:, m:m + 1])
                for l in range(2):
                    yn = [ph3.tile([128, 150], F32, tag=f"yn{l}_{m}", name=f"yn{l}_{m}") for m in range(2)]
                    for m in range(2):
                        ps = ps3.tile([128, 150], F32, tag="fps")
                        nc.tensor.matmul(ps[:], fwt[:, l, 0, m * 128:(m + 1) * 128], y[0][:],
                                         start=True, stop=False)
                        nc.tensor.matmul(ps[:], fwt[:, l, 1, m * 128:(m + 1) * 128], y[1][:],
                                         start=False, stop=True)
                        nc.scalar.activation(yn[m][:], ps[:], AF.Relu, bias=fbt[:, l, m:m + 1])
                    y = yn
                z3 = ph3.tile([64, 150], F32, tag="z3")
                ps = ps3.tile([128, 150], F32, tag="fps")
                nc.tensor.matmul(ps[0:64, :], fw3[:, 0, :], y[0][:], start=True, stop=False)
                nc.tensor.matmul(ps[0:64, :], fw3[:, 1, :], y[1][:], start=False, stop=True)
                nc.scalar.activation(z3[:], ps[0:64, :], AF.Relu, bias=fb3[:, 0:1])
                ps4 = ps3.tile([128, 150], F32, tag="fps")
                nc.tensor.matmul(ps4[0:1, :], fw4[:, 0:1], z3[:])
                score = ph3.tile([1, 150], F32, tag="score")
                nc.scalar.activation(score[:], ps4[0:1, :], AF.Sigmoid, bias=fb4[0:1, 0:1])
                dist = ph3.tile([1, 150], F32, tag="dist")
                nc.vector.tensor_scalar(dist[:], score[:], -1.0, 1.0, ALU.mult, ALU.add)
                if debug:
                    nc.sync.dma_start(dist_dbg_d[:], dist[:])

                # ---- margin loss (exact sorted(label*dist)[1] semantics) ----
                v = ph3.tile([1, 150], F32, tag="lv0")
                nc.vector.tensor_tensor(v[:], dist[:], lbl_sb[:], ALU.mult)
                vq = v.rearrange("p (q s) -> p q s", s=S)
                min1 = ph3.tile([1, 30], F32, tag="min1")
                nc.vector.tensor_reduce(min1[:], vq, axis=AX.X, op=ALU.min)
                eq = ph3.tile([1, 150], F32, tag="eq")
                nc.vector.tensor_tensor(eq.rearrange("p (q s) -> p q s", s=S), vq,
                                        min1[:, :, None].to_broadcast((1, 30, 5)), ALU.is_equal)
                cntg = ph3.tile([1, 30], F32, tag="cntg")  # 1.0 if >=2 mins tie
                nc.vector.reduce_sum(cntg[:], eq.rearrange("p (q s) -> p q s", s=S), axis=AX.X)
                nc.vector.tensor_scalar(cntg[:], cntg[:], 1.5, None, ALU.is_ge)
                vx = ph3.tile([1, 150], F32, tag="vx")
                nc.vector.tensor_scalar(vx[:], eq[:], 1e9, None, ALU.mult)
                nc.vector.tensor_tensor(vx[:], vx[:], v[:], ALU.add)
                excl = ph3.tile([1, 30], F32, tag="excl")
                nc.vector.tensor_reduce(excl[:], vx.rearrange("p (q s) -> p q s", s=S),
                                        axis=AX.X, op=ALU.min)
                # min_neg = cntg ? min1 : excl
                nsel = ph3.tile([1, 30], F32, tag="nsel")
                nc.vector.tensor_scalar(nsel[:], cntg[:], -1.0, 1.0, ALU.mult, ALU.add)
                mn = ph3.tile([1, 30], F32, tag="mn")
                nc.vector.tensor_tensor(mn[:], min1[:], cntg[:], ALU.mult)
                nc.vector.tensor_tensor(nsel[:], excl[:], nsel[:], ALU.mult)
                nc.vector.tensor_tensor(mn[:], mn[:], nsel[:], ALU.add)
                t2 = ph3.tile([1, 150], F32, tag="lt2")
                nc.vector.tensor_tensor(t2[:], dist[:], apmask_sb[:], ALU.mult)
                ap_ = ph3.tile([1, 30], F32, tag="ap")
                nc.vector.reduce_sum(ap_[:], t2.rearrange("p (q s) -> p q s", s=S), axis=AX.X)
                dd = ph3.tile([1, 30], F32, tag="dd")
                nc.vector.tensor_tensor(dd[:], ap_[:], mn[:], ALU.subtract)
                lv = ph3.tile([1, 30], F32, tag="lv")
                nc.scalar.activation(lv[:], dd[:], AF.Relu, bias=margin[0:1, 0:1])
                lp = ph3.tile([1, 1], F32, tag="lp")
                nc.vector.reduce_sum(lp[:], lv[:], axis=AX.X)
                nc.sync.dma_start(loss_d[:], lp[:])

    nc.compile()
    return nc


# ---------------------------------------------------------------------------
# host-side preparation
# ---------------------------------------------------------------------------

def _coord():
    ii = np.arange(3, dtype=np.float32) / 3.0
    c = np.stack([np.broadcast_to(ii[:, None], (3, 3)),
                  np.broadcast_to(ii[None, :], (3, 3))], 0).reshape(2, 9)
    return c


def make_in_maps(inp, n_cores=NCORES):
    p = {k: np.ascontiguousarray(np.asarray(v)) for k, v in inp.items()}
    coord = _coord()
    shared = {}
    w27 = p["w1"].transpose(2, 3, 1, 0).reshape(27, 64).astype(np.float32)
    w1t = np.zeros((54, 128), np.float32)
    w1t[0:27, 0:64] = w27; w1t[27:54, 64:128] = w27
    shared["w1t"] = w1t.astype(ml_dtypes.bfloat16)
    wct = np.stack([p["w2"], p["w3"], p["w4"]]).transpose(0, 3, 4, 2, 1).reshape(3, 9, 64, 64)
    wct = wct.transpose(2, 0, 1, 3)  # [ci, l, j, co]
    wbd = np.zeros((128, 3, 9, 128), np.float32)
    wbd[0:64, :, :, 0:64] = wct
    wbd[64:128, :, :, 64:128] = wct
    shared["wct"] = wbd
    shared["bng"] = np.stack([p[f"bn{i}_g"] for i in range(1, 5)], 1).astype(np.float32)
    shared["bnb"] = np.stack([p[f"bn{i}_b"] for i in range(1, 5)], 1).astype(np.float32)
    shared["gw1s"] = p["gw1"][:66].astype(np.float32)
    shared["gw1q"] = p["gw1"][66:].astype(np.float32)
    shared["gb1t"] = p["gb1"].reshape(2, 128).T.astype(np.float32)
    shared["gwt"] = np.stack([p["gw2"], p["gw3"], p["gw4"]]).reshape(3, 2, 128, 256).transpose(2, 0, 1, 3).astype(np.float32)
    shared["gbt"] = np.stack([p["gb2"], p["gb3"], p["gb4"]]).reshape(3, 2, 128).transpose(2, 0, 1).astype(np.float32)
    shared["fwt"] = np.stack([p["fw1"], p["fw2"]]).reshape(2, 2, 128, 256).transpose(2, 0, 1, 3).astype(np.float32)
    shared["fbt"] = np.stack([p["fb1"], p["fb2"]]).reshape(2, 2, 128).transpose(2, 0, 1).astype(np.float32)
    shared["fw3t"] = p["fw3"].reshape(2, 128, 64).transpose(1, 0, 2).astype(np.float32)
    shared["fb3t"] = p["fb3"].reshape(64, 1).astype(np.float32)
    shared["fw4t"] = p["fw4"].reshape(64, 1).astype(np.float32)
    shared["fb4t"] = p["fb4"].reshape(1, 1).astype(np.float32)
    shared["fbng"] = p["fbn_g"].reshape(2, 128).T.astype(np.float32)
    shared["fbnb"] = p["fbn_b"].reshape(2, 128).T.astype(np.float32)
    shared["coord45"] = np.tile(coord, (1, 5)).astype(np.float32)
    shared["coord270"] = np.tile(coord, (1, 30)).astype(np.float32)

    in_maps = []
    for c in range(n_cores):
        m = dict(shared)
        sup, qry = p["support_x"][c], p["query_x"][c]
        order = [sup[i] for i in range(5)] + [qry[i] for i in range(13)] \
            + [qry[13 + i] for i in range(17)] + [np.zeros((3, 84, 84), np.float32)]
        imgs = np.zeros((36, 3, PLANE), np.float32)
        imgs[:, :, :7056] = np.stack(order).reshape(36, 3, 7056)
        m["imgs"] = imgs.astype(ml_dtypes.bfloat16)
        same = (p["support_y"][c][None, :] == p["query_y"][c][:, None])
        m["lbl"] = (~same).astype(np.float32).reshape(1, 150)
        pos_idx = np.argmax(same, axis=1)
        apm = np.zeros((Q, S), np.float32)
        apm[np.arange(Q), pos_idx] = 1.0
        m["apmask"] = apm.reshape(1, 150)
        in_maps.append(m)
    return in_maps


_NC_CACHE = {}


def kernel(**inputs) -> np.ndarray:
    key = (NCORES, False)
    if key not in _NC_CACHE:
        _NC_CACHE[key] = build_nc(NCORES, debug=False)
    nc = _NC_CACHE[key]
    in_maps = make_in_maps(inputs, NCORES)
    res = run_bass_kernel_spmd(nc, in_maps, core_ids=list(range(NCORES)),
                               trace=bool(int(os.environ.get("KTRACE", "0"))))
    if res.exec_time_ns is not None:
        print(f"HW exec time: {res.exec_time_ns} ns")
    total = np.float64(sum(np.float64(r["loss_part"][0, 0]) for r in res.results))
    return np.asarray(total / NCORES, dtype=np.float32)


if __name__ == "__main__":
    d = np.load("/root/problem/ref_inputs.npz")
    inp = {k: d[k] for k in d.files}
    out = kernel(**inp)
    ref = np.load("/root/problem/ref_out.npy")
    print("kernel:", out, "ref:", ref, "rel err:", abs(out - ref) / max(abs(ref), 1e-12))


# revision 23
# speedup vs baseline: 1.0322x; 1.0322x over previous
"""Trainium2 Bass kernel for nn_Metric_42674795053594 (Relation Network loss).

Self-contained: hardcodes all shapes. Shards batch b=8 across 8 NeuronCores
(1 episode/core), replicates params, uses AllReduce for training-mode
BatchNorm statistics that couple all episodes.
"""
import sys, os
sys.path.insert(0, '/opt/trn_rl_repo')
import numpy as np
import ml_dtypes

import concourse.bass as bass
import concourse.mybir as mybir
import concourse.tile as tile
from concourse import bacc
from concourse.bass_utils import run_bass_kernel_spmd

F32 = mybir.dt.float32
BF16 = mybir.dt.bfloat16
AF = mybir.ActivationFunctionType
ALU = mybir.AluOpType
AX = mybir.AxisListType

EPS = 1e-5
NCORES = 8
S, Q = 5, 30
NPAIR = 18          # 36 image slots (5 sup + 30 qry + 1 pad) packed 2/partition-half
IMGW = 84
PLANE = 7232        # padded per-channel plane stride (>= 7056 + 170)
W1 = 7056           # conv1 output width (84*84)
PW1, PW2 = 1681, 361   # pooled widths: 41*41, 19*19
PAD1, PAD2 = 88, 40    # shift-overshoot pads (2*41+2+chunk slack, 2*19+2)
GROWS = Q * S * 81     # 12150 pairwise rows per core
CHUNK = 486            # 6 blocks of 81, <= 512

# conv2 input pooled1 is stored in this dtype (bf16 halves SBUF + 2x PE for conv2)
POOL1_DT = BF16


def _bn_scalar_ops(nc, pool, s_ap, q_ap, n_elems, g_ap, b_ap, sc_out, sh_out, eps_ap, tag):
    """Given sum (s_ap) and sumsq (q_ap) APs [P,1], counts, gamma/beta APs,
    write scale into sc_out and shift into sh_out ([P,1])."""
    P = s_ap.shape[0]
    t = pool.tile([128, 4], F32, tag=f"bns_{tag}")
    mean, ex2, var, m2 = t[:P, 0:1], t[:P, 1:2], t[:P, 2:3], t[:P, 3:4]
    nc.vector.tensor_scalar_mul(mean, s_ap, 1.0 / n_elems)
    nc.vector.tensor_scalar_mul(ex2, q_ap, 1.0 / n_elems)
    nc.vector.tensor_tensor(m2, mean, mean, ALU.mult)
    nc.vector.tensor_tensor(var, ex2, m2, ALU.subtract)
    # sd = sqrt(var + eps); inv = 1/sd
    nc.scalar.activation(var, var, AF.Sqrt, bias=eps_ap)
    nc.vector.reciprocal(var, var)
    nc.vector.tensor_tensor(sc_out, g_ap, var, ALU.mult)
    nc.vector.tensor_tensor(m2, mean, sc_out, ALU.mult)
    nc.vector.tensor_tensor(sh_out, b_ap, m2, ALU.subtract)


def build_nc(n_cores=NCORES, debug=False):
    nc = bacc.Bacc("TRN2", target_bir_lowering=False, debug=False, num_devices=n_cores)
    RG = [list(range(n_cores))]

    # ---------------- I/O ----------------
    imgs_d = nc.dram_tensor("imgs", [36, 3, PLANE], BF16, kind="ExternalInput")
    w1t_d = nc.dram_tensor("w1t", [54, 128], BF16, kind="ExternalInput")
    wct_d = nc.dram_tensor("wct", [128, 3, 9, 128], F32, kind="ExternalInput")
    bng_d = nc.dram_tensor("bng", [64, 4], F32, kind="ExternalInput")
    bnb_d = nc.dram_tensor("bnb", [64, 4], F32, kind="ExternalInput")
    gw1s_d = nc.dram_tensor("gw1s", [66, 256], F32, kind="ExternalInput")
    gw1q_d = nc.dram_tensor("gw1q", [66, 256], F32, kind="ExternalInput")
    gb1_d = nc.dram_tensor("gb1t", [128, 2], F32, kind="ExternalInput")
    gwt_d = nc.dram_tensor("gwt", [128, 3, 2, 256], F32, kind="ExternalInput")
    gbt_d = nc.dram_tensor("gbt", [128, 3, 2], F32, kind="ExternalInput")
    fwt_d = nc.dram_tensor("fwt", [128, 2, 2, 256], F32, kind="ExternalInput")
    fbt_d = nc.dram_tensor("fbt", [128, 2, 2], F32, kind="ExternalInput")
    fw3_d = nc.dram_tensor("fw3t", [128, 2, 64], F32, kind="ExternalInput")
    fb3_d = nc.dram_tensor("fb3t", [64, 1], F32, kind="ExternalInput")
    fw4_d = nc.dram_tensor("fw4t", [64, 1], F32, kind="ExternalInput")
    fb4_d = nc.dram_tensor("fb4t", [1, 1], F32, kind="ExternalInput")
    fbng_d = nc.dram_tensor("fbng", [128, 2], F32, kind="ExternalInput")
    fbnb_d = nc.dram_tensor("fbnb", [128, 2], F32, kind="ExternalInput")
    coord45_d = nc.dram_tensor("coord45", [2, 45], F32, kind="ExternalInput")
    coord270_d = nc.dram_tensor("coord270", [2, 270], F32, kind="ExternalInput")
    lbl_d = nc.dram_tensor("lbl", [1, 150], F32, kind="ExternalInput")
    apmask_d = nc.dram_tensor("apmask", [1, 150], F32, kind="ExternalInput")

    loss_d = nc.dram_tensor("loss_part", [1, 1], F32, kind="ExternalOutput")
    if debug:
        feats_dbg_d = nc.dram_tensor("feats_dbg", [66, 324], F32, kind="ExternalOutput")
        dist_dbg_d = nc.dram_tensor("dist_dbg", [1, 150], F32, kind="ExternalOutput")
        xf_dbg_d = nc.dram_tensor("xf_dbg", [128, 2, 150], F32, kind="ExternalOutput")

    with tile.TileContext(nc) as tc:
        with (
            tc.tile_pool(name="pers", bufs=1) as pers,
            tc.tile_pool(name="dram", bufs=1, space="DRAM") as dram,
        ):
            # ---------------- load persistent weights ----------------
            w1t = pers.tile([54, 128], BF16)
            nc.sync.dma_start(w1t[:], w1t_d[:])
            wct = pers.tile([128, 3, 9, 128], F32)
            nc.sync.dma_start(wct[:], wct_d[:])
            wct2b = pers.tile([128, 9, 128], POOL1_DT)   # conv2 weights in pooled1 dtype
            nc.vector.tensor_copy(wct2b[:], wct[:, 0])
            bng = pers.tile([64, 4], F32)
            nc.sync.dma_start(bng[:], bng_d[:])
            bnb = pers.tile([64, 4], F32)
            nc.sync.dma_start(bnb[:], bnb_d[:])
            gw1s = pers.tile([66, 256], F32)
            nc.sync.dma_start(gw1s[:], gw1s_d[:])
            gw1q = pers.tile([66, 256], F32)
            nc.sync.dma_start(gw1q[:], gw1q_d[:])
            gb1 = pers.tile([128, 2], F32)
            nc.sync.dma_start(gb1[:], gb1_d[:])
            gwt = pers.tile([128, 3, 2, 256], F32)
            nc.sync.dma_start(gwt[:], gwt_d[:])
            gbt = pers.tile([128, 3, 2], F32)
            nc.sync.dma_start(gbt[:], gbt_d[:])
            fwt = pers.tile([128, 2, 2, 256], F32)
            nc.sync.dma_start(fwt[:], fwt_d[:])
            fbt = pers.tile([128, 2, 2], F32)
            nc.sync.dma_start(fbt[:], fbt_d[:])
            fw3 = pers.tile([128, 2, 64], F32)
            nc.sync.dma_start(fw3[:], fw3_d[:])
            fb3 = pers.tile([64, 1], F32)
            nc.sync.dma_start(fb3[:], fb3_d[:])
            fw4 = pers.tile([64, 1], F32)
            nc.sync.dma_start(fw4[:], fw4_d[:])
            fb4 = pers.tile([1, 1], F32)
            nc.sync.dma_start(fb4[:], fb4_d[:])
            fbng = pers.tile([128, 2], F32)
            nc.sync.dma_start(fbng[:], fbng_d[:])
            fbnb = pers.tile([128, 2], F32)
            nc.sync.dma_start(fbnb[:], fbnb_d[:])
            lbl_sb = pers.tile([1, 150], F32)
            nc.sync.dma_start(lbl_sb[:], lbl_d[:])
            apmask_sb = pers.tile([1, 150], F32)
            nc.sync.dma_start(apmask_sb[:], apmask_d[:])

            epsc = pers.tile([128, 1], F32)
            nc.gpsimd.memset(epsc[:], EPS)
            margin = pers.tile([1, 1], F32)
            nc.gpsimd.memset(margin[:], 0.2)

            # persistent activations
            pooled2 = pers.tile([128, NPAIR * PW2 + PAD2], F32)
            nc.gpsimd.memset(pooled2[:, NPAIR * PW2:], 0.0)
            feats = pers.tile([66, 324], F32)
            nc.sync.dma_start(feats[64:66, 0:45], coord45_d[:])
            nc.sync.dma_start(feats[64:66, 45:315], coord270_d[:])
            # BN scale/shift per conv layer: [128, 2] (col0 sup, col1 qry)
            sc_t = [pers.tile([128, 2], F32, tag=f"sc{l}", name=f"sc{l}") for l in range(4)]
            sh_t = [pers.tile([128, 2], F32, tag=f"sh{l}", name=f"sh{l}") for l in range(4)]
            xf = pers.tile([128, 2, 150], F32)

            # ---- helper: stats + allreduce + scale/shift for one conv layer ----
            def conv_bn(layer, buf, Wimg, valid_view_fn, sup_elems, qry_elems, cc_tag, sum_axis=AX.X):
                """buf: [128, NPAIR*Wimg(+pad)]; valid_view_fn(half_slice, p0, np_)
                returns the valid-region AP for pairs [p0, p0+np_).
                Computes region sums + per-pair sumsq, allreduces, fills
                sc_t[layer], sh_t[layer]."""
                st = pers.tile([128, 8], F32, tag=f"stt{layer}")
                # per-pair sums + sumsq (overlap with conv); region-combine later
                sumacc = pers.tile([128, NPAIR], F32, tag=f"sma{layer}")
                sqacc = pers.tile([128, NPAIR], F32, tag=f"sqa{layer}")
                sqs = pers.tile([128, 512], F32, tag="sq_scratch")
                for p in range(NPAIR):
                    v = valid_view_fn(slice(0, 128), p, 1)
                    nc.vector.reduce_sum(sumacc[:, p:p + 1], v, axis=sum_axis)
                    n_el = v.free_size()
                    # chunk the square dump through the 512-wide scratch
                    if n_el <= 512:
                        nc.scalar.activation(sqs[:, :n_el], v, AF.Square,
                                             accum_out=sqacc[:, p:p + 1])
                    else:
                        # pooled maps are contiguous; square in 512-col chunks
                        flat = buf[:, p * Wimg:(p + 1) * Wimg]
                        nsub = (Wimg + 511) // 512
                        part = pers.tile([128, 4], F32, tag=f"sqp{layer}")
                        for sub in range(nsub):
                            a, b = sub * 512, min((sub + 1) * 512, Wimg)
                            nc.scalar.activation(sqs[:, :b - a], flat[:, a:b], AF.Square,
                                                 accum_out=part[:, sub % 4:sub % 4 + 1])
                        # sum the partials (nsub<=4)
                        nc.vector.reduce_sum(sqacc[:, p:p + 1], part[:, :nsub], axis=AX.X)
                nc.vector.reduce_sum(st[0:64, 0:1], sumacc[0:64, 0:5], axis=AX.X)
                nc.vector.reduce_sum(st[0:64, 1:2], sumacc[0:64, 5:18], axis=AX.X)
                nc.vector.reduce_sum(st[64:128, 1:2], sumacc[64:128, :], axis=AX.X)
                nc.vector.reduce_sum(st[0:64, 2:3], sqacc[0:64, 0:5], axis=AX.X)
                nc.vector.reduce_sum(st[0:64, 3:4], sqacc[0:64, 5:18], axis=AX.X)
                nc.vector.reduce_sum(st[64:128, 3:4], sqacc[64:128, :], axis=AX.X)
                # pack [64, 4]: sup_sum, sup_sq, qry_sum_partial(top), qry_sq_partial(top)
                pk = pers.tile([64, 8], F32, tag=f"pk{layer}")
                nc.vector.tensor_copy(pk[:, 0:1], st[0:64, 0:1])
                nc.vector.tensor_copy(pk[:, 1:2], st[0:64, 2:3])
                # qry partials: top + bottom (bottom moved down via DMA)
                nc.sync.dma_start(pk[:, 4:5], st[64:128, 1:2])
                nc.sync.dma_start(pk[:, 5:6], st[64:128, 3:4])
                nc.vector.tensor_tensor(pk[:, 2:3], st[0:64, 1:2], pk[:, 4:5], ALU.add)
                nc.vector.tensor_tensor(pk[:, 3:4], st[0:64, 3:4], pk[:, 5:6], ALU.add)
                # allgather [64,4] -> [64*N,4], then local sum (AG floor < AR floor)
                bin_ = dram.tile([64, 4], F32, tag=f"ccin{cc_tag}")
                bout = dram.tile([64 * n_cores, 4], F32, tag=f"ccout{cc_tag}")
                nc.gpsimd.dma_start(bin_[:], pk[:, 0:4])
                nc.gpsimd.collective_compute("AllGather", ALU.bypass, replica_groups=RG,
                                             ins=[bin_.opt()], outs=[bout.opt()])
                gat = pers.tile([64, 4 * n_cores], F32, tag=f"gat{layer}")
                nc.sync.dma_start(gat[:], bout.rearrange("(r p) f -> p r f", p=64))
                red = pers.tile([64, 4], F32, tag=f"red{layer}")
                nc.vector.reduce_sum(red[:], gat.rearrange("p (r f) -> p f r", r=n_cores),
                                     axis=AX.X)
                _bn_scalar_ops(nc, pers, red[:, 0:1], red[:, 1:2], sup_elems,
                               bng[:, layer:layer + 1], bnb[:, layer:layer + 1],
                               sc_t[layer][0:64, 0:1], sh_t[layer][0:64, 0:1], epsc[0:64], f"s{layer}")
                _bn_scalar_ops(nc, pers, red[:, 2:3], red[:, 3:4], qry_elems,
                               bng[:, layer:layer + 1], bnb[:, layer:layer + 1],
                               sc_t[layer][0:64, 1:2], sh_t[layer][0:64, 1:2], epsc[0:64], f"q{layer}")
                # replicate qry scale/shift to bottom partitions
                nc.sync.dma_start(sc_t[layer][64:128, 1:2], sc_t[layer][0:64, 1:2])
                nc.sync.dma_start(sh_t[layer][64:128, 1:2], sh_t[layer][0:64, 1:2])

            def bn_apply_pairs(layer, view_fn, out_view_fn=None):
                # per-pair BN+relu so the next conv layer pipelines behind it
                for p in range(NPAIR):
                    top_in = view_fn(slice(0, 64), p, 1)
                    bot_in = view_fn(slice(64, 128), p, 1)
                    top_out = out_view_fn(slice(0, 64), p, 1) if out_view_fn else top_in
                    bot_out = out_view_fn(slice(64, 128), p, 1) if out_view_fn else bot_in
                    col = 0 if p < 5 else 1  # top half holds support for pairs 0-4
                    nc.scalar.activation(top_out, top_in, AF.Relu,
                                         bias=sh_t[layer][0:64, col:col + 1],
                                         scale=sc_t[layer][0:64, col:col + 1])
                    nc.scalar.activation(bot_out, bot_in, AF.Relu,
                                         bias=sh_t[layer][64:128, 1:2],
                                         scale=sc_t[layer][64:128, 1:2])

            # ================= PHASE 1: conv1 + pool + BN, conv2 + pool + BN =================
            with (
                tc.tile_pool(name="ph1", bufs=1) as ph1,
                tc.tile_pool(name="ph1b", bufs=3) as ph1b,
                tc.tile_pool(name="ph1ps", bufs=6, space="PSUM") as psum,
            ):
                pooled1 = ph1.tile([128, NPAIR * PW1 + PAD1], POOL1_DT)
                nc.gpsimd.memset(pooled1[:, NPAIR * PW1:], 0.0)

                for p in range(NPAIR):
                    in27 = ph1b.tile([54, W1], BF16, tag="in27")
                    for half, img in ((0, p), (1, 18 + p)):
                        for kx in range(3):
                            src_ap = bass.AP(tensor=imgs_d.ap().tensor,
                                             offset=img * 3 * PLANE + kx * IMGW,
                                             ap=[[1, 3], [PLANE, 3], [1, W1]])
                            r0 = half * 27 + kx * 9
                            nc.sync.dma_start(in27[r0:r0 + 9, :], src_ap)
                    # 14 chunks of 504 cols = 6 input rows each; pool 2x2 from PSUM
                    for c in range(14):
                        a = c * 504
                        w = 504 if c < 13 else 336
                        inr = 6 if c < 13 else 4
                        ps = psum.tile([128, 512], F32, tag="cps")
                        nc.tensor.matmul(ps[:, :w], w1t[:, :], in27[:, a:a + w])
                        orows = 3 if c < 13 else 2
                        v5 = ps[:, :w].rearrange("p (r c) -> p r c", r=inr)[:, :, 0:82] \
                            .rearrange("p (R rp) (C cp) -> p R C rp cp", rp=2, cp=2)[:, :orows]
                        dst = pooled1[:, p * PW1 + 3 * c * 41: p * PW1 + (3 * c + orows) * 41]
                        nc.vector.tensor_reduce(dst.rearrange("p (r c) -> p r c", r=orows),
                                                v5, axis=AX.XY, op=ALU.max)

                # ---- L1 BN ----
                def l1_view(hs, p0, np_):
                    return pooled1[hs, p0 * PW1:(p0 + np_) * PW1]
                conv_bn(0, pooled1, PW1, l1_view, NCORES * 5 * PW1, NCORES * 30 * PW1, 0)
                bn_apply_pairs(0, l1_view)
                nc.gpsimd.memset(pooled1[64:128, 17 * PW1:18 * PW1], 0.0)  # pad img

                # ---- conv2 (bf16 in, fp32 psum) + pool (41->39 valid ->19) ----
                c2widths = [492, 492, 492, 82]
                for p in range(NPAIR):
                    base = p * PW1
                    pstiles = [psum.tile([128, 512], F32, tag="cps", name=f"c2ps{_i}") for _i in range(4)]
                    for j in range(9):
                        sh = (j // 3) * 41 + (j % 3)
                        for c in range(4):
                            a = c * 492
                            w = c2widths[c]
                            nc.tensor.matmul(
                                pstiles[c][:, :w], wct2b[:, j, :],
                                pooled1[:, base + a + sh: base + a + sh + w],
                                start=(j == 0), stop=(j == 8))
                    for c in range(4):
                        orows = 6 if c < 3 else 1
                        inrows = 12 if c < 3 else 2
                        v5 = pstiles[c][:, :inrows * 41].rearrange("p (r c) -> p r c", r=inrows)[:, :2 * orows, 0:38] \
                            .rearrange("p (R rp) (C cp) -> p R C rp cp", rp=2, cp=2)
                        dst = pooled2[:, p * PW2 + 6 * c * 19: p * PW2 + (6 * c + orows) * 19]
                        nc.vector.tensor_reduce(dst.rearrange("p (r c) -> p r c", r=orows),
                                                v5, axis=AX.XY, op=ALU.max)

            # ---- L2 BN ----
            def l2_view(hs, p0, np_):
                return pooled2[hs, p0 * PW2:(p0 + np_) * PW2]
            conv_bn(1, pooled2, PW2, l2_view, NCORES * 5 * PW2, NCORES * 30 * PW2, 1)
            bn_apply_pairs(1, l2_view)
            nc.gpsimd.memset(pooled2[64:128, 17 * PW2:18 * PW2], 0.0)

            # ================= PHASE 2: conv3, conv4, avgpool =================
            with (
                tc.tile_pool(name="ph2", bufs=1) as ph2,
                tc.tile_pool(name="ph2ps", bufs=8, space="PSUM") as psum,
            ):
                c3buf = ph2.tile([128, NPAIR * PW2 + PAD2], F32)
                nc.gpsimd.memset(c3buf[:, NPAIR * PW2:], 0.0)
                PW3 = 289  # 17*17 repacked width for conv4
                c17 = ph2.tile([128, NPAIR * PW3 + 36], F32)
                nc.gpsimd.memset(c17[:, NPAIR * PW3:], 0.0)
                c4buf = ph2.tile([128, NPAIR * PW3], F32)

                def conv_layer(src, dstbuf, lidx, W, Wo, Wc):
                    # src [128, NPAIR*W(+pad)]; dst stride Wo; compute only Wc cols
                    for pb in range(0, NPAIR, 4):
                        pe = min(pb + 4, NPAIR)
                        pst = {pp: psum.tile([128, 512], F32, tag="cps", name=f"c34ps{pp}") for pp in range(pb, pe)}
                        rowlen = int(round(W ** 0.5))
                        for j in range(9):
                            sh = (j // 3) * rowlen + (j % 3)
                            for pp in range(pb, pe):
                                base = pp * W
                                nc.tensor.matmul(
                                    pst[pp][:, :Wc], wct[:, lidx, j, :],
                                    src[:, base + sh: base + sh + Wc],
                                    start=(j == 0), stop=(j == 8))
                        for pp in range(pb, pe):
                            nc.scalar.activation(dstbuf[:, pp * Wo:pp * Wo + Wc],
                                                 pst[pp][:, :Wc], AF.Copy)

                conv_layer(pooled2, c3buf, 1, PW2, PW2, 17 * 19)

                def l3_view(hs, p0, np_):
                    return c3buf[hs, p0 * PW2:(p0 + np_) * PW2].rearrange(
                        "p (i r c) -> p i r c", r=19, c=19)[:, :, 0:17, 0:17]
                def c17_view(hs, p0, np_):
                    return c17[hs, p0 * PW3:(p0 + np_) * PW3].rearrange(
                        "p (i r c) -> p i r c", r=17, c=17)
                conv_bn(2, c3buf, PW2, l3_view, NCORES * 5 * 289, NCORES * 30 * 289, 2, sum_axis=AX.XYZ)
                bn_apply_pairs(2, l3_view, c17_view)
                nc.gpsimd.memset(c17[64:128, 17 * PW3:18 * PW3], 0.0)

                conv_layer(c17, c4buf, 2, PW3, PW3, 15 * 17)

                def l4_view(hs, p0, np_):
                    return c4buf[hs, p0 * PW3:(p0 + np_) * PW3].rearrange(
                        "p (i r c) -> p i r c", r=17, c=17)[:, :, 0:15, 0:15]
                conv_bn(3, c4buf, PW3, l4_view, NCORES * 5 * 225, NCORES * 30 * 225, 3, sum_axis=AX.XYZ)
                bn_apply_pairs(3, l4_view)

                # ---- avgpool 5x5 -> [64, 9] per image ----
                featsB = ph2.tile([128, 162], F32)
                ptmp = ph2.tile([128, 45], F32, tag="ptmp")
                for i in range(36):
                    half, p = (0, i) if i < 18 else (1, i - 18)
                    if half == 1 and p == 17:
                        continue  # pad image unused
                    hs = slice(half * 64, half * 64 + 64)
                    base = p * PW3
                    # stage 1: sum over kcol(5): [64, 15(17), 3(5), 5(1)] -> [64, 45]
                    v1 = c4buf[hs, base:base + PW3].rearrange(
                        "p (r c) -> p r c", r=17)[:, 0:15, 0:15].rearrange(
                        "p r (oc k) -> p r oc k", oc=3)
                    nc.vector.reduce_sum(ptmp[hs, :].rearrange("p (r oc) -> p r oc", r=15),
                                         v1, axis=AX.X)
                    # stage 2: sum over krow(5): t[r, oc] r=5R+kr -> [64, 9]
                    v2 = ptmp[hs, :].rearrange("p (R k oc) -> p R oc k", R=3, k=5, oc=3)
                    if half == 0:
                        dst = feats[0:64, i * 9:(i + 1) * 9].rearrange("p (R oc) -> p R oc", R=3)
                        nc.vector.reduce_sum(dst, v2, axis=AX.X)
                    else:
                        dstB = featsB[hs, p * 9:(p + 1) * 9].rearrange("p (R oc) -> p R oc", R=3)
                        nc.vector.reduce_sum(dstB, v2, axis=AX.X)
                nc.sync.dma_start(feats[0:64, 162:315], featsB[64:128, 0:153])
                nc.vector.tensor_scalar_mul(feats[0:64, 0:315], feats[0:64, 0:315], 1.0 / 25.0)

            if debug:
                nc.sync.dma_start(feats_dbg_d[:], feats[:])

            # ================= PHASE 3: pairwise g-MLP + f-MLP + loss =================
            with (
                tc.tile_pool(name="ph3", bufs=3) as ph3,
                tc.tile_pool(name="ph3psum", bufs=2, space="PSUM") as ps3,
                tc.tile_pool(name="ph3psg", bufs=2, space="PSUM") as psg,
            ):
                # A[mb] [128, 45], B[mb] [128, 270]
                A = [ph3.tile([128, 45], F32, tag=f"A{m}", name=f"A{m}") for m in range(2)]
                Bq = [ph3.tile([128, 270], F32, tag=f"B{m}", name=f"B{m}") for m in range(2)]
                for m in range(2):
                    pa = ps3.tile([128, 512], F32, tag="abps")
                    nc.tensor.matmul(pa[:, 0:45], gw1s[:, m * 128:(m + 1) * 128], feats[:, 0:45])
                    nc.scalar.activation(A[m][:], pa[:, 0:45], AF.Identity, bias=gb1[:, m:m + 1])
                    pb = ps3.tile([128, 512], F32, tag="abps")
                    nc.tensor.matmul(pb[:, 0:270], gw1q[:, m * 128:(m + 1) * 128], feats[:, 45:315])
                    nc.scalar.activation(Bq[m][:], pb[:, 0:270], AF.Copy)

                QCH = 405  # one query row-block: 5 s * 81 xy
                for qp in range(0, Q, 2):
                    qpair = (qp, qp + 1)
                    h = {}
                    for qi, q in enumerate(qpair):
                        x1 = [ph3.tile([128, QCH], F32, tag=f"x1_{qi}_{k}", name=f"x1_{qi}_{k}")
                              for k in range(2)]
                        for k in range(2):
                            a_in = A[k][:, :, None].to_broadcast((128, 45, 9))
                            b_in = Bq[k][:, None, q * 9:q * 9 + 9].to_broadcast((128, 45, 9))
                            out = x1[k][:].rearrange("p (sx y) -> p sx y", y=9)
                            nc.vector.tensor_tensor(out, a_in, b_in, ALU.add)
                            nc.scalar.activation(x1[k][:], x1[k][:], AF.Relu)
                        h[qi] = x1
                    for l in range(3):
                        hn = {qi: [ph3.tile([128, QCH], F32, tag=f"h{qi}_{l}_{m}", name=f"h{qi}_{l}_{m}")
                                   for m in range(2)] for qi in range(2)}
                        for m in range(2):
                            ps = {qi: psg.tile([128, 512], F32, tag=f"gps{qi}", name=f"gps{qi}")
                                  for qi in range(2)}
                            for ks in range(2):
                                for qi in range(2):
                                    nc.tensor.matmul(ps[qi][:, :QCH],
                                                     gwt[:, l, ks, m * 128:(m + 1) * 128],
                                                     h[qi][ks][:],
                                                     start=(ks == 0), stop=(ks == 1))
                            for qi in range(2):
                                nc.scalar.activation(hn[qi][m][:], ps[qi][:, :QCH], AF.Relu,
                                                     bias=gbt[:, l, m:m + 1])
                        h = hn
                    for qi, q in enumerate(qpair):
                        for m in range(2):
                            nc.vector.reduce_sum(xf[:, m, q * 5:(q + 1) * 5],
                                                 h[qi][m].rearrange("p (b e) -> p b e", e=81), axis=AX.X)

                # ---- fbn stats + allreduce ----
                fst = ph3.tile([128, 4], F32, tag="fst")
                sqf = ph3.tile([128, 150], F32, tag="sqf")
                for m in range(2):
                    nc.vector.reduce_sum(fst[:, 2 * m:2 * m + 1], xf[:, m], axis=AX.X)
                    nc.scalar.activation(sqf[:], xf[:, m], AF.Square,
                                         accum_out=fst[:, 2 * m + 1:2 * m + 2])
                fbin = dram.tile([128, 4], F32, tag="ccfin")
                fbout = dram.tile([128 * n_cores, 4], F32, tag="ccfout")
                nc.gpsimd.dma_start(fbin[:], fst[:])
                nc.gpsimd.collective_compute("AllGather", ALU.bypass, replica_groups=RG,
                                             ins=[fbin.opt()], outs=[fbout.opt()])
                fgat = ph3.tile([128, 4 * n_cores], F32, tag="fgat")
                nc.sync.dma_start(fgat[:], fbout.rearrange("(r p) f -> p r f", p=128))
                fred = ph3.tile([128, 4], F32, tag="fred")
                nc.vector.reduce_sum(fred[:], fgat.rearrange("p (r f) -> p f r", r=n_cores),
                                     axis=AX.X)
                fsc = ph3.tile([128, 2], F32, tag="fsc")
                fsh = ph3.tile([128, 2], F32, tag="fsh")
                for m in range(2):
                    _bn_scalar_ops(nc, ph3, fred[:, 2 * m:2 * m + 1], fred[:, 2 * m + 1:2 * m + 2],
                                   1200.0, fbng[:, m:m + 1], fbnb[:, m:m + 1],
                                   fsc[:, m:m + 1], fsh[:, m:m + 1], epsc[:], f"f{m}")

                if debug:
                    nc.sync.dma_start(xf_dbg_d[:], xf[:])

                # ---- f-MLP on [*, 150] ----
                y = [ph3.tile([128, 150], F32, tag=f"y{m}", name=f"y{m}") for m in range(2)]
                for m in range(2):
                    nc.scalar.activation(y[m][:], xf[:, m], AF.Identity,
                                         bias=fsh[:, m:m + 1], scale=fsc[:, m:m + 1])
                for l in range(2):
                    yn = [ph3.tile([128, 150], F32, tag=f"yn{l}_{m}", name=f"yn{l}_{m}") for m in range(2)]
                    for m in range(2):
                        ps = ps3.tile([128, 150], F32, tag="fps")
                        nc.tensor.matmul(ps[:], fwt[:, l, 0, m * 128:(m + 1) * 128], y[0][:],
                                         start=True, stop=False)
                        nc.tensor.matmul(ps[:], fwt[:, l, 1, m * 128:(m + 1) * 128], y[1][:],
                                         start=False, stop=True)
                        nc.scalar.activation(yn[m][:], ps[:], AF.Relu, bias=fbt[:, l, m:m + 1])
                    y = yn
                z3 = ph3.tile([64, 150], F32, tag="z3")
                ps = ps3.tile([128, 150], F32, tag="fps")
                nc.tensor.matmul(ps[0:64, :], fw3[:, 0, :], y[0][:], start=True, stop=False)
                nc.tensor.matmul(ps[0:64, :], fw3[:, 1, :], y[1][:], start=False, stop=True)
                nc.scalar.activation(z3[:], ps[0:64, :], AF.Relu, bias=fb3[:, 0:1])
                ps4 = ps3.tile([128, 150], F32, tag="fps")
                nc.tensor.matmul(ps4[0:1, :], fw4[:, 0:1], z3[:])
                score = ph3.tile([1, 150], F32, tag="score")
                nc.scalar.activation(score[:], ps4[0:1, :], AF.Sigmoid, bias=fb4[0:1, 0:1])
                dist = ph3.tile([1, 150], F32, tag="dist")
                nc.vector.tensor_scalar(dist[:], score[:], -1.0, 1.0, ALU.mult, ALU.add)
                if debug:
                    nc.sync.dma_start(dist_dbg_d[:], dist[:])

                # ---- margin loss (exact sorted(label*dist)[1] semantics) ----
                v = ph3.tile([1, 150], F32, tag="lv0")
                nc.vector.tensor_tensor(v[:], dist[:], lbl_sb[:], ALU.mult)
                vq = v.rearrange("p (q s) -> p q s", s=S)
                min1 = ph3.tile([1, 30], F32, tag="min1")
                nc.vector.tensor_reduce(min1[:], vq, axis=AX.X, op=ALU.min)
                eq = ph3.tile([1, 150], F32, tag="eq")
                nc.vector.tensor_tensor(eq.rearrange("p (q s) -> p q s", s=S), vq,
                                        min1[:, :, None].to_broadcast((1, 30, 5)), ALU.is_equal)
                cntg = ph3.tile([1, 30], F32, tag="cntg")  # 1.0 if >=2 mins tie
                nc.vector.reduce_sum(cntg[:], eq.rearrange("p (q s) -> p q s", s=S), axis=AX.X)
                nc.vector.tensor_scalar(cntg[:], cntg[:], 1.5, None, ALU.is_ge)
                vx = ph3.tile([1, 150], F32, tag="vx")
                nc.vector.tensor_scalar(vx[:], eq[:], 1e9, None, ALU.mult)
                nc.vector.tensor_tensor(vx[:], vx[:], v[:], ALU.add)
                excl = ph3.tile([1, 30], F32, tag="excl")
                nc.vector.tensor_reduce(excl[:], vx.rearrange("p (q s) -> p q s", s=S),
                                        axis=AX.X, op=ALU.min)
                # min_neg = cntg ? min1 : excl
                nsel = ph3.tile([1, 30], F32, tag="nsel")
                nc.vector.tensor_scalar(nsel[:], cntg[:], -1.0, 1.0, ALU.mult, ALU.add)
                mn = ph3.tile([1, 30], F32, tag="mn")
                nc.vector.tensor_tensor(mn[:], min1[:], cntg[:], ALU.mult)
                nc.vector.tensor_tensor(nsel[:], excl[:], nsel[:], ALU.mult)
                nc.vector.tensor_tensor(mn[:], mn[:], nsel[:], ALU.add)
                t2 = ph3.tile([1, 150], F32, tag="lt2")
                nc.vector.tensor_tensor(t2[:], dist[:], apmask_sb[:], ALU.mult)
                ap_ = ph3.tile([1, 30], F32, tag="ap")
                nc.vector.reduce_sum(ap_[:], t2.rearrange("p (q s) -> p q s", s=S), axis=AX.X)
                dd = ph3.tile([1, 30], F32, tag="dd")
                nc.vector.tensor_tensor(dd[:], ap_[:], mn[:], ALU.subtract)
                lv = ph3.tile([1, 30], F32, tag="lv")
                nc.scalar.activation(lv[:], dd[:], AF.Relu, bias=margin[0:1, 0:1])
                lp = ph3.tile([1, 1], F32, tag="lp")
                nc.vector.reduce_sum(lp[:], lv[:], axis=AX.X)
                nc.sync.dma_start(loss_d[:], lp[:])

    nc.compile()
    return nc


# ---------------------------------------------------------------------------
# host-side preparation
# ---------------------------------------------------------------------------

def _coord():
    ii = np.arange(3, dtype=np.float32) / 3.0
    c = np.stack([np.broadcast_to(ii[:, None], (3, 3)),
                  np.broadcast_to(ii[None, :], (3, 3))], 0).reshape(2, 9)
    return c


def make_in_maps(inp, n_cores=NCORES):
    p = {k: np.ascontiguousarray(np.asarray(v)) for k, v in inp.items()}
    coord = _coord()
    shared = {}
    w27 = p["w1"].transpose(2, 3, 1, 0).reshape(27, 64).astype(np.float32)
    w1t = np.zeros((54, 128), np.float32)
    w1t[0:27, 0:64] = w27; w1t[27:54, 64:128] = w27
    shared["w1t"] = w1t.astype(ml_dtypes.bfloat16)
    wct = np.stack([p["w2"], p["w3"], p["w4"]]).transpose(0, 3, 4, 2, 1).reshape(3, 9, 64, 64)
    wct = wct.transpose(2, 0, 1, 3)  # [ci, l, j, co]
    wbd = np.zeros((128, 3, 9, 128), np.float32)
    wbd[0:64, :, :, 0:64] = wct
    wbd[64:128, :, :, 64:128] = wct
    shared["wct"] = wbd
    shared["bng"] = np.stack([p[f"bn{i}_g"] for i in range(1, 5)], 1).astype(np.float32)
    shared["bnb"] = np.stack([p[f"bn{i}_b"] for i in range(1, 5)], 1).astype(np.float32)
    shared["gw1s"] = p["gw1"][:66].astype(np.float32)
    shared["gw1q"] = p["gw1"][66:].astype(np.float32)
    shared["gb1t"] = p["gb1"].reshape(2, 128).T.astype(np.float32)
    shared["gwt"] = np.stack([p["gw2"], p["gw3"], p["gw4"]]).reshape(3, 2, 128, 256).transpose(2, 0, 1, 3).astype(np.float32)
    shared["gbt"] = np.stack([p["gb2"], p["gb3"], p["gb4"]]).reshape(3, 2, 128).transpose(2, 0, 1).astype(np.float32)
    shared["fwt"] = np.stack([p["fw1"], p["fw2"]]).reshape(2, 2, 128, 256).transpose(2, 0, 1, 3).astype(np.float32)
    shared["fbt"] = np.stack([p["fb1"], p["fb2"]]).reshape(2, 2, 128).transpose(2, 0, 1).astype(np.float32)
    shared["fw3t"] = p["fw3"].reshape(2, 128, 64).transpose(1, 0, 2).astype(np.float32)
    shared["fb3t"] = p["fb3"].reshape(64, 1).astype(np.float32)
    shared["fw4t"] = p["fw4"].reshape(64, 1).astype(np.float32)
    shared["fb4t"] = p["fb4"].reshape(1, 1).astype(np.float32)
    shared["fbng"] = p["fbn_g"].reshape(2, 128).T.astype(np.float32)
    shared["fbnb"] = p["fbn_b"].reshape(2, 128).T.astype(np.float32)
    shared["coord45"] = np.tile(coord, (1, 5)).astype(np.float32)
    shared["coord270"] = np.tile(coord, (1, 30)).astype(np.float32)

    in_maps = []
    for c in range(n_cores):
        m = dict(shared)
        sup, qry = p["support_x"][c], p["query_x"][c]
        order = [sup[i] for i in range(5)] + [qry[i] for i in range(13)] \
            + [qry[13 + i] for i in range(17)] + [np.zeros((3, 84, 84), np.float32)]
        imgs = np.zeros((36, 3, PLANE), np.float32)
        imgs[:, :, :7056] = np.stack(order).reshape(36, 3, 7056)
        m["imgs"] = imgs.astype(ml_dtypes.bfloat16)
        same = (p["support_y"][c][None, :] == p["query_y"][c][:, None])
        m["lbl"] = (~same).astype(np.float32).reshape(1, 150)
        pos_idx = np.argmax(same, axis=1)
        apm = np.zeros((Q, S), np.float32)
        apm[np.arange(Q), pos_idx] = 1.0
        m["apmask"] = apm.reshape(1, 150)
        in_maps.append(m)
    return in_maps


_NC_CACHE = {}


def kernel(**inputs) -> np.ndarray:
    key = (NCORES, False)
    if key not in _NC_CACHE:
        _NC_CACHE[key] = build_nc(NCORES, debug=False)
    nc = _NC_CACHE[key]
    in_maps = make_in_maps(inputs, NCORES)
    res = run_bass_kernel_spmd(nc, in_maps, core_ids=list(range(NCORES)),
                               trace=bool(int(os.environ.get("KTRACE", "0"))))
    if res.exec_time_ns is not None:
        print(f"HW exec time: {res.exec_time_ns} ns")
    total = np.float64(sum(np.float64(r["loss_part"][0, 0]) for r in res.results))
    return np.asarray(total / NCORES, dtype=np.float32)


if __name__ == "__main__":
    d = np.load("/root/problem/ref_inputs.npz")
    inp = {k: d[k] for k in d.files}
    out = kernel(**inp)
    ref = np.load("/root/problem/ref_out.npy")
    print("kernel:", out, "ref:", ref, "rel err:", abs(out - ref) / max(abs(ref), 1e-12))
